# revision 28
# baseline (speedup 1.0000x reference)
"""Trainium2 Bass kernel for the HMM forward-algorithm problem.

Strategy
--------
The reference does, per time step, a log-domain matrix-vector product
  alpha_t[b,k] = em[b,t,k] + logsumexp_j(alpha_{t-1}[b,j] + tran[j,k])
followed by logsumexp_k.  We run the whole recurrence in *probability*
domain on the TensorEngine:

  phat_t = E_t  *  (phat_{t-1} @ P)          (elementwise * matmul)

where P = softmax(tran) rows (constant) and E_t = exp(em_t - kappa) with a
global shift kappa that keeps E <= ~1.  phat decays by ~e^-3 per step, so we
renormalise every RN steps by an earlier column sum (dumping the exact f32
scale used so the host can undo it).

The recurrence is a T-link serial chain PE -> (PSUM latency) -> DVE multiply
-> (latency) -> PE whose per-link latency is fixed-cost dominated, so the 8
batch rows per core are split into TWO independent 4-row chains that
interleave: each chain's link is cheaper and the engines stay busy with the
other chain during latency gaps.  Everything else is kept OFF the chains:

- renorm: the reciprocal/broadcast/E-scale are prepared 5+ steps ahead and
  folded into a pre-scaled E-strip slice, so renorm steps cost nothing;
- per-step column sums (the per-t logsumexp output) accumulate into a PSUM
  strip of RN slots, copied out by the Act engine once per RN steps;
- emission gathers: indirect DMA fetches bf16 rows two blocks ahead; the 4
  sources are summed via matmul-by-identity transposes accumulating in PSUM
  (PE idle windows), then Act applies exp(0.25*x - L - kappa) into the
  E-strip.

Emissions: em[b,t,h] = 0.25 * sum_s x[s,h,obs[b,t,s]] - L[h], where
x is the raw emission table and L[h] = 0.25*sum_s logsumexp_v x[s,h,:].
The host pre-transposes x to a (S*V, H) bf16 row table; the device gathers
rows with indirect DMA (128 rows = 16 timesteps x 8 batch per source).

Sharding: data-parallel over batch (8 of 64 rows per core).  Tables are
replicated.  No collectives.  Final log / cumsum / length-indexing is tiny
(T x B) and done on the host in float64.
"""
import sys

sys.path.insert(0, "/opt/trn_rl_repo")

import numpy as np
import ml_dtypes

import concourse.bass as bass
import concourse.bacc as bacc
import concourse.tile as tile
import concourse.mybir as mybir
import concourse.bass_utils as bass_utils
from concourse.masks import make_identity

B, T, S, H, V = 64, 512, 4, 512, 10000
NC = 8            # cores
BL = B // NC      # batch rows per core
NG = 2            # independent chains per core
BG = BL // NG     # batch rows per chain
P_ = 128          # partitions
HCN = H // P_     # h chunks
TBLK = 16         # timesteps per gather block
RN = 13           # renorm interval
F32 = mybir.dt.float32
BF16 = mybir.dt.bfloat16
I32 = mybir.dt.int32
EXP = mybir.ActivationFunctionType.Exp
MULT = mybir.AluOpType.mult

_compiled = {}


def _n_renorms(t_steps):
    return len([t for t in range(1, t_steps) if t % RN == 0])


def build(t_steps=T):
    """Build + bacc-compile the per-core Bass program (identical on all cores)."""
    nblk = t_steps // TBLK
    nc = bacc.Bacc("TRN2", target_bir_lowering=False, debug=False,
                   enable_asserts=False, num_devices=NC)

    tabt = nc.dram_tensor("tabt", [S * V, H], BF16, kind="ExternalInput").ap()
    pm_d = nc.dram_tensor("pm", [P_, HCN * HCN * P_], BF16, kind="ExternalInput").ap()
    idx_d = nc.dram_tensor("idx", [P_, S * nblk], I32, kind="ExternalInput").ap()
    bias_d = nc.dram_tensor("bias", [P_, HCN], F32, kind="ExternalInput").ap()
    expp_d = nc.dram_tensor("expp", [P_, HCN], F32, kind="ExternalInput").ap()
    rstrip_d = nc.dram_tensor("rstrip", [1, t_steps * BL], F32,
                              kind="ExternalOutput").ap()
    nrn = max(1, _n_renorms(t_steps))
    rinv_d = nc.dram_tensor("rinvstrip", [1, nrn * BL], F32,
                            kind="ExternalOutput").ap()

    with tile.TileContext(nc) as tc:
        with (tc.tile_pool(name="const", bufs=1) as cp,
              tc.tile_pool(name="estrip", bufs=nblk) as ep,
              tc.tile_pool(name="gath", bufs=12) as gp,
              tc.tile_pool(name="phat", bufs=3) as pp,
              tc.tile_pool(name="small", bufs=2) as sp,
              tc.tile_pool(name="ebr", bufs=2) as er,
              tc.tile_pool(name="qpsum", bufs=2, space="PSUM") as qp,
              tc.tile_pool(name="rstripps", bufs=2, space="PSUM") as rp,
              tc.tile_pool(name="combops", bufs=1, space="PSUM") as cbp,
              tc.tile_pool(name="tpsum", bufs=1, space="PSUM") as tp_):

            # ---- constants ----
            idx_t = cp.tile([P_, S * nblk], I32, name="idxt")
            nc.sync.dma_start(idx_t[:, :], idx_d[:, :])
            pm_t = cp.tile([P_, HCN * HCN * P_], BF16, name="pmt")
            nc.sync.dma_start(pm_t[:, :], pm_d[:, :])
            bias_t = cp.tile([P_, HCN], F32, name="biast")
            nc.sync.dma_start(bias_t[:, :], bias_d[:, :])
            expp_t = cp.tile([P_, HCN], F32, name="exppt")
            nc.sync.dma_start(expp_t[:, :], expp_d[:, :])
            ones128 = cp.tile([P_, 1], BF16, name="ones128")
            nc.gpsimd.memset(ones128[:, :], 1.0)
            onesrow_f = cp.tile([1, P_], F32, name="onesrowf")
            nc.gpsimd.memset(onesrow_f[:, :], 1.0)
            identb = cp.tile([P_, P_], BF16, name="identb")
            make_identity(nc, identb[:, :])
            rstrip_t = cp.tile([1, t_steps * BL], F32, name="rstript")
            rinv_t = cp.tile([1, nrn * BL], F32, name="rinvt")

            eb_list = [None] * nblk
            g_list = [None] * nblk

            def emit_gather(blk, idx_ap=None, idx_stride=None):
                gs = []
                for s in range(S):
                    g = gp.tile([P_, H], BF16, tag="g", name=f"g{blk}_{s}")
                    if idx_ap is None:
                        off = idx_t[:, s * nblk + blk:s * nblk + blk + 1]
                    else:
                        off = idx_ap[:, s:s + 1]
                    nc.gpsimd.indirect_dma_start(
                        out=g[:, :], out_offset=None, in_=tabt[:, :],
                        in_offset=bass.IndirectOffsetOnAxis(ap=off, axis=0))
                    gs.append(g)
                g_list[blk] = gs
                eb_list[blk] = ep.tile([P_, TBLK * HCN * BL], BF16, tag="eb",
                                       name=f"eb{blk}")

            def emit_chunk(blk, c):
                # transpose the 4 source gathers for h-chunk c, summing in
                # PSUM, then exp into the E-strip on the Act engine
                gs = g_list[blk]
                tpp = tp_.tile([P_, P_], F32, tag="tp")
                for s in range(S):
                    nc.tensor.matmul(tpp[:, :],
                                     lhsT=gs[s][:, c * P_:(c + 1) * P_],
                                     rhs=identb[:, :],
                                     start=(s == 0), stop=(s == S - 1))
                eb4 = eb_list[blk].rearrange("p (t c b) -> p t c b",
                                             t=TBLK, c=HCN)
                nc.scalar.activation(
                    eb4[:, :, c, :],
                    tpp.rearrange("p (t b) -> p t b", t=TBLK),
                    EXP, bias=bias_t[:, c:c + 1], scale=0.25)
                return tpp

            def eb_slice(t, g):
                # [128, (HCN, BG)] E-strip view for chain g at step t
                eb4 = eb_list[t // TBLK].rearrange("p (t c b) -> p t c b",
                                                   t=TBLK, c=HCN)
                return eb4[:, t % TBLK, :, g * BG:(g + 1) * BG]

            # ---- block 0: gathers, transposes, E-strip, phat_0 ----
            emit_gather(0)
            phat = [pp.tile([P_, HCN * BG], BF16, tag=f"ph{g}",
                            name=f"phat0_{g}") for g in range(NG)]
            tpp0 = [tp_.tile([P_, P_], F32, tag="tp", name="tpp0_0"),
                    qp.tile([P_, P_], F32, tag="q0", name="tpp0_1"),
                    qp.tile([P_, P_], F32, tag="q1", name="tpp0_2"),
                    cbp.tile([P_, P_], F32, tag="combo", name="tpp0_3")]
            for s_ in range(S):
                for c in range(HCN):
                    nc.tensor.matmul(tpp0[c][:, :],
                                     lhsT=g_list[0][s_][:, c * P_:(c + 1) * P_],
                                     rhs=identb[:, :],
                                     start=(s_ == 0), stop=(s_ == S - 1))
            eb4_0 = eb_list[0].rearrange("p (t c b) -> p t c b", t=TBLK, c=HCN)
            for c in range(HCN):
                nc.scalar.activation(
                    eb4_0[:, :, c, :],
                    tpp0[c].rearrange("p (t b) -> p t b", t=TBLK),
                    EXP, bias=bias_t[:, c:c + 1], scale=0.25)
                for g in range(NG):
                    nc.vector.tensor_scalar_mul(
                        phat[g][:, c * BG:(c + 1) * BG],
                        eb4_0[:, 0, c, g * BG:(g + 1) * BG],
                        expp_t[:, c:c + 1])
            idx1_t = cp.tile([P_, S], I32, name="idx1t")
            iv = idx_t.rearrange("p (s n) -> p s n", s=S)
            nc.scalar.copy(idx1_t[:, :], iv[:, :, 1])
            emit_gather(1, idx_ap=idx1_t)

            # ---- interleaved gather + two-chain scan ----
            # combo PSUM tile columns: rb_g at [g*16:(g+1)*16), r2_g at
            # [32+g*4 : 32+(g+1)*4) on partition 0
            ridx = 0
            rps = None
            combo = None
            tiled = None
            rv8 = None
            ebr_cur = [None, None]
            last_rn = (t_steps - 1) // RN * RN  # last renorm step < t_steps
            CW = HCN * BG                      # rb width per chain (16)

            def rgroup(g, u):
                # column sums of chain g's phat_u into PSUM r-strip slot u%RN
                nonlocal rps
                if u % RN == 0 and g == 0:
                    rps = rp.tile([1, RN * BL], F32, tag="rstrip")
                lo = (u % RN) * BL + g * BG
                for jc in range(HCN):
                    nc.tensor.matmul(rps[:, lo:lo + BG],
                                     lhsT=ones128[:, :],
                                     rhs=phat[g][:, jc * BG:(jc + 1) * BG],
                                     start=(jc == 0), stop=(jc == HCN - 1))

            for t in range(1, t_steps):
                blk = t // TBLK
                j = t % TBLK
                m = t % RN
                tr = t - m + RN          # next renorm step after t
                prep = (m >= RN - 6 and tr <= last_rn)

                # PE: q_g = P^T phat_g (16 matmuls each), then column sums
                qs = []
                for g in range(NG):
                    q = qp.tile([P_, HCN * BG], F32, tag=f"q{g}")
                    for kc in range(HCN):
                        for jc in range(HCN):
                            nc.tensor.matmul(
                                q[:, kc * BG:(kc + 1) * BG],
                                lhsT=pm_t[:, (jc * HCN + kc) * P_:
                                          (jc * HCN + kc + 1) * P_],
                                rhs=phat[g][:, jc * BG:(jc + 1) * BG],
                                start=(jc == 0), stop=(jc == HCN - 1))
                    qs.append(q)
                    rgroup(g, t - 1)
                if (t - 1) % RN == RN - 1:
                    grp = (t - 1) // RN
                    nc.scalar.copy(
                        rstrip_t[:, grp * RN * BL:(grp + 1) * RN * BL],
                        rps[:, :])
                    # (full groups only inside the loop)
                    if grp == (t_steps - 1) // RN - 1:
                        nc.sync.dma_start(
                            rstrip_d[:, :(grp + 1) * RN * BL],
                            rstrip_t[:, :(grp + 1) * RN * BL])
                # PE (off-chain): renorm scale source = column sums of phat
                if prep and m == RN - 6:
                    combo = cbp.tile([P_, NG * CW + NG * BG], F32, tag="combo")
                    for g in range(NG):
                        lo = NG * CW + g * BG
                        for jc in range(HCN):
                            nc.tensor.matmul(
                                combo[0:1, lo:lo + BG], lhsT=ones128[:, :],
                                rhs=phat[g][:, jc * BG:(jc + 1) * BG],
                                start=(jc == 0), stop=(jc == HCN - 1))
                # PE (off-chain): broadcast rinv over partitions
                if prep and m == RN - 3:
                    for g in range(NG):
                        nc.tensor.matmul(combo[:, g * CW:(g + 1) * CW],
                                         lhsT=onesrow_f[:, :],
                                         rhs=tiled[:, g * CW:(g + 1) * CW],
                                         start=True, stop=True)
                # Pool: prefetch gathers two blocks ahead
                if j == 14 and blk + 2 < nblk:
                    emit_gather(blk + 2)
                # PE/Act (off-chain): transpose+exp bursts for next block
                if blk + 1 < nblk and 10 <= j <= 13:
                    emit_chunk(blk + 1, j - 10)

                # DVE: the chain multiplies
                for g in range(NG):
                    pnew = pp.tile([P_, HCN * BG], BF16, tag=f"ph{g}")
                    pv = pnew.rearrange("p (c b) -> p c b", c=HCN)
                    qv = qs[g].rearrange("p (c b) -> p c b", c=HCN)
                    if m == 0 and ebr_cur[g] is not None:
                        ev = ebr_cur[g].rearrange(
                            "p (c b) -> p c b", c=HCN)[:, :, g * BG:(g + 1) * BG]
                        ebr_cur[g] = None
                    else:
                        ev = eb_slice(t, g)
                    nc.vector.tensor_tensor(pv[:, :, :], qv[:, :, :],
                                            ev[:, :, :], MULT)
                    phat[g] = pnew
                if t == t_steps - 6:
                    nc.sync.dma_start(rinv_d[:, :], rinv_t[:, :])
                if prep and m == RN - 2:
                    ebr = er.tile([P_, HCN * BL], BF16, tag="ebr")
                    cv = combo[:, 0:NG * CW].rearrange(
                        "p (g c b) -> p c g b", g=NG, c=HCN)
                    eb4r = eb_list[tr // TBLK].rearrange(
                        "p (t c b) -> p t c b", t=TBLK, c=HCN)
                    e4 = eb4r[:, tr % TBLK, :, :].rearrange(
                        "p c (g b) -> p c g b", g=NG)
                    o4 = ebr.rearrange("p (c g b) -> p c g b", c=HCN, g=NG)
                    nc.vector.tensor_tensor(o4[:, :, :, :], e4[:, :, :, :],
                                            cv[:, :, :, :], MULT)
                    ebr_cur = [ebr, ebr]

                # DVE/Act (off-chain): renorm preparation pipeline
                if prep and m == RN - 5:
                    rv8 = sp.tile([1, BL], F32, tag="rv8")
                    nc.vector.reciprocal(rv8[:, :],
                                         combo[0:1, NG * CW:NG * CW + BL])
                    nc.scalar.copy(rinv_t[:, ridx * BL:(ridx + 1) * BL],
                                   rv8[:, :])
                    ridx += 1
                    tiled = sp.tile([1, NG * CW], F32, tag="tiled")
                    for g in range(NG):
                        o = g * CW
                        nc.scalar.copy(tiled[:, o:o + BG],
                                       rv8[:, g * BG:(g + 1) * BG])
                        nc.scalar.copy(tiled[:, o + BG:o + 2 * BG],
                                       tiled[:, o:o + BG])
                        nc.scalar.copy(tiled[:, o + 2 * BG:o + 4 * BG],
                                       tiled[:, o:o + 2 * BG])

            for g in range(NG):
                rgroup(g, t_steps - 1)
            grp = (t_steps - 1) // RN
            w = (t_steps - grp * RN) * BL
            nc.scalar.copy(rstrip_t[:, grp * RN * BL:grp * RN * BL + w],
                           rps[:, 0:w])
            flo = ((t_steps - 1) // RN) * RN * BL
            nc.sync.dma_start(rstrip_d[:, flo:], rstrip_t[:, flo:])
            if t_steps <= 6:
                nc.sync.dma_start(rinv_d[:, :], rinv_t[:, :])

    nc.compile()
    return nc


def _get_compiled(t_steps=T):
    if t_steps not in _compiled:
        _compiled[t_steps] = build(t_steps)
    return _compiled[t_steps]


def _host_prep(obs, emis, tran, priors, t_steps):
    """Returns (shared_inputs, per_core_idx, kappa)."""
    nblk = t_steps // TBLK
    # transition softmax -> bf16 chunk layout [j, (jc*HCN+kc)*128 + k]
    m = tran.max(axis=1, keepdims=True)
    e = np.exp(tran - m, dtype=np.float32)
    P = (e / e.sum(axis=1, keepdims=True)).astype(ml_dtypes.bfloat16)
    pm = np.ascontiguousarray(
        P.reshape(HCN, P_, HCN, P_).transpose(1, 0, 2, 3).reshape(P_, -1))

    # transposed bf16 emission table, rows indexed by s*V+v
    tabT = np.ascontiguousarray(
        emis.transpose(0, 2, 1)).astype(ml_dtypes.bfloat16).reshape(S * V, H)

    # L[h] and kappa
    mx = emis.max(axis=2)                                   # (S,H)
    lse = mx + np.log(np.exp(emis - mx[:, :, None],
                             dtype=np.float32).sum(axis=2))
    L = 0.25 * lse.sum(axis=0)                              # (H,)
    kap_h = 0.25 * mx.sum(axis=0) - L
    kappa = float(kap_h.max())
    bias = np.ascontiguousarray(
        (-(L + kappa)).astype(np.float32).reshape(HCN, P_).T)   # (128,4)
    expp = np.ascontiguousarray(
        np.exp(priors, dtype=np.float32).reshape(HCN, P_).T)

    # per-core gather row indices: idx[p=(tt*BL+bb), s*nblk+blk]
    per_core_idx = []
    svec = (np.arange(S, dtype=np.int64) * V)
    for c in range(NC):
        o = obs[c * BL:(c + 1) * BL, :t_steps, :]           # (BL,t,S)
        o = o + svec[None, None, :]
        o = o.transpose(1, 0, 2)                            # (t, BL, S)
        o = o.reshape(nblk, TBLK, BL, S)
        o = o.transpose(1, 2, 3, 0).reshape(TBLK * BL, S * nblk)
        per_core_idx.append(np.ascontiguousarray(o.astype(np.int32)))

    shared = {"tabt": tabT, "pm": pm, "bias": bias, "expp": expp}
    return shared, per_core_idx, kappa


def _host_post(results, lengths, kappa, t_steps):
    nrn = max(1, _n_renorms(t_steps))
    ans = np.zeros((B, 1), np.float32)
    tt = np.arange(t_steps, dtype=np.float64)
    for c in range(NC):
        r = results[c]["rstrip"].reshape(t_steps, BL).astype(np.float64)
        rinv = results[c]["rinvstrip"].reshape(nrn, BL).astype(np.float64)
        rho_log = np.zeros((t_steps, BL), np.float64)
        k = 0
        for t in range(1, t_steps):
            if t % RN == 0:
                rho_log[t] = np.log(rinv[k])
                k += 1
        logsums = np.log(r) + (tt[:, None] + 1.0) * kappa \
            - np.cumsum(rho_log, axis=0)
        lens = np.clip(lengths[c * BL:(c + 1) * BL], 1, t_steps)
        ans[c * BL:(c + 1) * BL, 0] = logsums[
            lens - 1, np.arange(BL)].astype(np.float32)
    return ans


def run(inputs, t_steps=T, trace=False):
    obs = np.asarray(inputs["obs"])
    lengths = np.asarray(inputs["lengths"])
    emis = np.asarray(inputs["unnormalized_emis"], np.float32)
    tran = np.asarray(inputs["unnormalized_tran"], np.float32)
    priors = np.asarray(inputs["log_state_priors"], np.float32)

    nc = _get_compiled(t_steps)
    shared, per_core_idx, kappa = _host_prep(obs, emis, tran, priors, t_steps)
    in_maps = [dict(shared, idx=per_core_idx[c]) for c in range(NC)]
    res = bass_utils.run_bass_kernel_spmd(nc, in_maps,
                                          core_ids=list(range(NC)),
                                          trace=trace)
    ans = _host_post(res.results, lengths, kappa, t_steps)
    return ans, res


def kernel(obs, lengths, unnormalized_emis, unnormalized_tran,
           log_state_priors):
    ans, _ = run(dict(obs=obs, lengths=lengths,
                      unnormalized_emis=unnormalized_emis,
                      unnormalized_tran=unnormalized_tran,
                      log_state_priors=log_state_priors))
    return ans


# revision 29
# speedup vs baseline: 1.0041x; 1.0041x over previous
"""Trainium2 Bass kernel for the HMM forward-algorithm problem.

Strategy
--------
The reference does, per time step, a log-domain matrix-vector product
  alpha_t[b,k] = em[b,t,k] + logsumexp_j(alpha_{t-1}[b,j] + tran[j,k])
followed by logsumexp_k.  We run the whole recurrence in *probability*
domain on the TensorEngine:

  phat_t = E_t  *  (phat_{t-1} @ P)          (elementwise * matmul)

where P = softmax(tran) rows (constant) and E_t = exp(em_t - kappa) with a
global shift kappa that keeps E <= ~1.  phat decays by ~e^-3 per step, so we
renormalise every RN steps by an earlier column sum (dumping the exact f32
scale used so the host can undo it).

The recurrence is a T-link serial chain PE -> (PSUM latency) -> DVE multiply
-> (latency) -> PE whose per-link latency is fixed-cost dominated, so the 8
batch rows per core are split into TWO independent 4-row chains that
interleave: each chain's link is cheaper and the engines stay busy with the
other chain during latency gaps.  Everything else is kept OFF the chains:

- renorm: the reciprocal/broadcast/E-scale are prepared 5+ steps ahead and
  folded into a pre-scaled E-strip slice, so renorm steps cost nothing;
- per-step column sums (the per-t logsumexp output) accumulate into a PSUM
  strip of RN slots, copied out by the Act engine once per RN steps;
- emission gathers: indirect DMA fetches bf16 rows two blocks ahead; the 4
  sources are summed via matmul-by-identity transposes accumulating in PSUM
  (PE idle windows), then Act applies exp(0.25*x - L - kappa) into the
  E-strip.

Emissions: em[b,t,h] = 0.25 * sum_s x[s,h,obs[b,t,s]] - L[h], where
x is the raw emission table and L[h] = 0.25*sum_s logsumexp_v x[s,h,:].
The host pre-transposes x to a (S*V, H) bf16 row table; the device gathers
rows with indirect DMA (128 rows = 16 timesteps x 8 batch per source).

Sharding: data-parallel over batch (8 of 64 rows per core).  Tables are
replicated.  No collectives.  Final log / cumsum / length-indexing is tiny
(T x B) and done on the host in float64.
"""
import sys

sys.path.insert(0, "/opt/trn_rl_repo")

import numpy as np
import ml_dtypes

import concourse.bass as bass
import concourse.bacc as bacc
import concourse.tile as tile
import concourse.mybir as mybir
import concourse.bass_utils as bass_utils
from concourse.masks import make_identity

B, T, S, H, V = 64, 512, 4, 512, 10000
NC = 8            # cores
BL = B // NC      # batch rows per core
NG = 2            # independent chains per core
BG = BL // NG     # batch rows per chain
P_ = 128          # partitions
HCN = H // P_     # h chunks
TBLK = 16         # timesteps per gather block
RN = 12           # renorm interval
F32 = mybir.dt.float32
BF16 = mybir.dt.bfloat16
I32 = mybir.dt.int32
EXP = mybir.ActivationFunctionType.Exp
MULT = mybir.AluOpType.mult

_compiled = {}


def _n_renorms(t_steps):
    return len([t for t in range(1, t_steps) if t % RN == 0])


def build(t_steps=T):
    """Build + bacc-compile the per-core Bass program (identical on all cores)."""
    nblk = t_steps // TBLK
    nc = bacc.Bacc("TRN2", target_bir_lowering=False, debug=False,
                   enable_asserts=False, num_devices=NC)

    tabt = nc.dram_tensor("tabt", [S * V, H], BF16, kind="ExternalInput").ap()
    pm_d = nc.dram_tensor("pm", [P_, HCN * HCN * P_], BF16, kind="ExternalInput").ap()
    idx_d = nc.dram_tensor("idx", [P_, S * nblk], I32, kind="ExternalInput").ap()
    bias_d = nc.dram_tensor("bias", [P_, HCN], F32, kind="ExternalInput").ap()
    expp_d = nc.dram_tensor("expp", [P_, HCN], F32, kind="ExternalInput").ap()
    rstrip_d = nc.dram_tensor("rstrip", [1, t_steps * BL], F32,
                              kind="ExternalOutput").ap()
    nrn = max(1, _n_renorms(t_steps))
    rinv_d = nc.dram_tensor("rinvstrip", [1, nrn * BL], F32,
                            kind="ExternalOutput").ap()

    with tile.TileContext(nc) as tc:
        with (tc.tile_pool(name="const", bufs=1) as cp,
              tc.tile_pool(name="estrip", bufs=nblk) as ep,
              tc.tile_pool(name="gath", bufs=12) as gp,
              tc.tile_pool(name="phat", bufs=3) as pp,
              tc.tile_pool(name="small", bufs=2) as sp,
              tc.tile_pool(name="ebr", bufs=2) as er,
              tc.tile_pool(name="qpsum", bufs=2, space="PSUM") as qp,
              tc.tile_pool(name="rstripps", bufs=2, space="PSUM") as rp,
              tc.tile_pool(name="combops", bufs=1, space="PSUM") as cbp,
              tc.tile_pool(name="tpsum", bufs=1, space="PSUM") as tp_):

            # ---- constants ----
            idx_t = cp.tile([P_, S * nblk], I32, name="idxt")
            nc.sync.dma_start(idx_t[:, :], idx_d[:, :])
            pm_t = cp.tile([P_, HCN * HCN * P_], BF16, name="pmt")
            nc.sync.dma_start(pm_t[:, :], pm_d[:, :])
            bias_t = cp.tile([P_, HCN], F32, name="biast")
            nc.sync.dma_start(bias_t[:, :], bias_d[:, :])
            expp_t = cp.tile([P_, HCN], F32, name="exppt")
            nc.sync.dma_start(expp_t[:, :], expp_d[:, :])
            ones128 = cp.tile([P_, 1], BF16, name="ones128")
            nc.gpsimd.memset(ones128[:, :], 1.0)
            onesrow_f = cp.tile([1, P_], F32, name="onesrowf")
            nc.gpsimd.memset(onesrow_f[:, :], 1.0)
            identb = cp.tile([P_, P_], BF16, name="identb")
            make_identity(nc, identb[:, :])
            rstrip_t = cp.tile([1, t_steps * BL], F32, name="rstript")
            rinv_t = cp.tile([1, nrn * BL], F32, name="rinvt")

            eb_list = [None] * nblk
            g_list = [None] * nblk

            def emit_gather(blk, idx_ap=None, idx_stride=None):
                gs = []
                for s in range(S):
                    g = gp.tile([P_, H], BF16, tag="g", name=f"g{blk}_{s}")
                    if idx_ap is None:
                        off = idx_t[:, s * nblk + blk:s * nblk + blk + 1]
                    else:
                        off = idx_ap[:, s:s + 1]
                    nc.gpsimd.indirect_dma_start(
                        out=g[:, :], out_offset=None, in_=tabt[:, :],
                        in_offset=bass.IndirectOffsetOnAxis(ap=off, axis=0))
                    gs.append(g)
                g_list[blk] = gs
                eb_list[blk] = ep.tile([P_, TBLK * HCN * BL], BF16, tag="eb",
                                       name=f"eb{blk}")

            def emit_chunk(blk, c):
                # transpose the 4 source gathers for h-chunk c, summing in
                # PSUM, then exp into the E-strip on the Act engine
                gs = g_list[blk]
                tpp = tp_.tile([P_, P_], F32, tag="tp")
                for s in range(S):
                    nc.tensor.matmul(tpp[:, :],
                                     lhsT=gs[s][:, c * P_:(c + 1) * P_],
                                     rhs=identb[:, :],
                                     start=(s == 0), stop=(s == S - 1))
                eb4 = eb_list[blk].rearrange("p (t c b) -> p t c b",
                                             t=TBLK, c=HCN)
                nc.scalar.activation(
                    eb4[:, :, c, :],
                    tpp.rearrange("p (t b) -> p t b", t=TBLK),
                    EXP, bias=bias_t[:, c:c + 1], scale=0.25)
                return tpp

            def eb_slice(t, g):
                # [128, (HCN, BG)] E-strip view for chain g at step t
                eb4 = eb_list[t // TBLK].rearrange("p (t c b) -> p t c b",
                                                   t=TBLK, c=HCN)
                return eb4[:, t % TBLK, :, g * BG:(g + 1) * BG]

            # ---- block 0: gathers, transposes, E-strip, phat_0 ----
            emit_gather(0)
            phat = [pp.tile([P_, HCN * BG], BF16, tag=f"ph{g}",
                            name=f"phat0_{g}") for g in range(NG)]
            tpp0 = [tp_.tile([P_, P_], F32, tag="tp", name="tpp0_0"),
                    qp.tile([P_, P_], F32, tag="q0", name="tpp0_1"),
                    qp.tile([P_, P_], F32, tag="q1", name="tpp0_2"),
                    cbp.tile([P_, P_], F32, tag="combo", name="tpp0_3")]
            for s_ in range(S):
                for c in range(HCN):
                    nc.tensor.matmul(tpp0[c][:, :],
                                     lhsT=g_list[0][s_][:, c * P_:(c + 1) * P_],
                                     rhs=identb[:, :],
                                     start=(s_ == 0), stop=(s_ == S - 1))
            eb4_0 = eb_list[0].rearrange("p (t c b) -> p t c b", t=TBLK, c=HCN)
            for c in range(HCN):
                nc.scalar.activation(
                    eb4_0[:, :, c, :],
                    tpp0[c].rearrange("p (t b) -> p t b", t=TBLK),
                    EXP, bias=bias_t[:, c:c + 1], scale=0.25)
                for g in range(NG):
                    nc.vector.tensor_scalar_mul(
                        phat[g][:, c * BG:(c + 1) * BG],
                        eb4_0[:, 0, c, g * BG:(g + 1) * BG],
                        expp_t[:, c:c + 1])
            idx1_t = cp.tile([P_, S], I32, name="idx1t")
            iv = idx_t.rearrange("p (s n) -> p s n", s=S)
            nc.scalar.copy(idx1_t[:, :], iv[:, :, 1])
            emit_gather(1, idx_ap=idx1_t)

            # ---- interleaved gather + two-chain scan ----
            # combo PSUM tile columns: rb_g at [g*16:(g+1)*16), r2_g at
            # [32+g*4 : 32+(g+1)*4) on partition 0
            ridx = 0
            rps = None
            combo = None
            tiled = None
            rv8 = None
            ebr_cur = [None, None]
            last_rn = (t_steps - 1) // RN * RN  # last renorm step < t_steps
            CW = HCN * BG                      # rb width per chain (16)

            def rgroup(g, u):
                # column sums of chain g's phat_u into PSUM r-strip slot u%RN
                nonlocal rps
                if u % RN == 0 and g == 0:
                    rps = rp.tile([1, RN * BL], F32, tag="rstrip")
                lo = (u % RN) * BL + g * BG
                for jc in range(HCN):
                    nc.tensor.matmul(rps[:, lo:lo + BG],
                                     lhsT=ones128[:, :],
                                     rhs=phat[g][:, jc * BG:(jc + 1) * BG],
                                     start=(jc == 0), stop=(jc == HCN - 1))

            for t in range(1, t_steps):
                blk = t // TBLK
                j = t % TBLK
                m = t % RN
                tr = t - m + RN          # next renorm step after t
                prep = (m >= RN - 6 and tr <= last_rn)

                # PE: q_g = P^T phat_g (16 matmuls each), then column sums
                qs = []
                for g in range(NG):
                    q = qp.tile([P_, HCN * BG], F32, tag=f"q{g}")
                    for kc in range(HCN):
                        for jc in range(HCN):
                            nc.tensor.matmul(
                                q[:, kc * BG:(kc + 1) * BG],
                                lhsT=pm_t[:, (jc * HCN + kc) * P_:
                                          (jc * HCN + kc + 1) * P_],
                                rhs=phat[g][:, jc * BG:(jc + 1) * BG],
                                start=(jc == 0), stop=(jc == HCN - 1))
                    qs.append(q)
                    rgroup(g, t - 1)
                if (t - 1) % RN == RN - 1:
                    grp = (t - 1) // RN
                    nc.scalar.copy(
                        rstrip_t[:, grp * RN * BL:(grp + 1) * RN * BL],
                        rps[:, :])
                    # (full groups only inside the loop)
                    if grp == (t_steps - 1) // RN - 1:
                        nc.sync.dma_start(
                            rstrip_d[:, :(grp + 1) * RN * BL],
                            rstrip_t[:, :(grp + 1) * RN * BL])
                # PE (off-chain): renorm scale source = column sums of phat
                if prep and m == RN - 6:
                    combo = cbp.tile([P_, NG * CW + NG * BG], F32, tag="combo")
                    for g in range(NG):
                        lo = NG * CW + g * BG
                        for jc in range(HCN):
                            nc.tensor.matmul(
                                combo[0:1, lo:lo + BG], lhsT=ones128[:, :],
                                rhs=phat[g][:, jc * BG:(jc + 1) * BG],
                                start=(jc == 0), stop=(jc == HCN - 1))
                # PE (off-chain): broadcast rinv over partitions
                if prep and m == RN - 3:
                    for g in range(NG):
                        nc.tensor.matmul(combo[:, g * CW:(g + 1) * CW],
                                         lhsT=onesrow_f[:, :],
                                         rhs=tiled[:, g * CW:(g + 1) * CW],
                                         start=True, stop=True)
                # Pool: prefetch gathers two blocks ahead
                if j == 14 and blk + 2 < nblk:
                    emit_gather(blk + 2)
                # PE/Act (off-chain): transpose+exp bursts for next block
                if blk + 1 < nblk and 10 <= j <= 13:
                    emit_chunk(blk + 1, j - 10)

                # DVE: the chain multiplies
                for g in range(NG):
                    pnew = pp.tile([P_, HCN * BG], BF16, tag=f"ph{g}")
                    pv = pnew.rearrange("p (c b) -> p c b", c=HCN)
                    qv = qs[g].rearrange("p (c b) -> p c b", c=HCN)
                    if m == 0 and ebr_cur[g] is not None:
                        ev = ebr_cur[g].rearrange(
                            "p (c b) -> p c b", c=HCN)[:, :, g * BG:(g + 1) * BG]
                        ebr_cur[g] = None
                    else:
                        ev = eb_slice(t, g)
                    nc.vector.tensor_tensor(pv[:, :, :], qv[:, :, :],
                                            ev[:, :, :], MULT)
                    phat[g] = pnew
                if t == t_steps - 6:
                    nc.sync.dma_start(rinv_d[:, :], rinv_t[:, :])
                if prep and m == RN - 2:
                    ebr = er.tile([P_, HCN * BL], BF16, tag="ebr")
                    cv = combo[:, 0:NG * CW].rearrange(
                        "p (g c b) -> p c g b", g=NG, c=HCN)
                    eb4r = eb_list[tr // TBLK].rearrange(
                        "p (t c b) -> p t c b", t=TBLK, c=HCN)
                    e4 = eb4r[:, tr % TBLK, :, :].rearrange(
                        "p c (g b) -> p c g b", g=NG)
                    o4 = ebr.rearrange("p (c g b) -> p c g b", c=HCN, g=NG)
                    nc.vector.tensor_tensor(o4[:, :, :, :], e4[:, :, :, :],
                                            cv[:, :, :, :], MULT)
                    ebr_cur = [ebr, ebr]

                # DVE/Act (off-chain): renorm preparation pipeline
                if prep and m == RN - 5:
                    rv8 = sp.tile([1, BL], F32, tag="rv8")
                    nc.vector.reciprocal(rv8[:, :],
                                         combo[0:1, NG * CW:NG * CW + BL])
                    nc.scalar.copy(rinv_t[:, ridx * BL:(ridx + 1) * BL],
                                   rv8[:, :])
                    ridx += 1
                    tiled = sp.tile([1, NG * CW], F32, tag="tiled")
                    for g in range(NG):
                        o = g * CW
                        nc.scalar.copy(tiled[:, o:o + BG],
                                       rv8[:, g * BG:(g + 1) * BG])
                        nc.scalar.copy(tiled[:, o + BG:o + 2 * BG],
                                       tiled[:, o:o + BG])
                        nc.scalar.copy(tiled[:, o + 2 * BG:o + 4 * BG],
                                       tiled[:, o:o + 2 * BG])

            for g in range(NG):
                rgroup(g, t_steps - 1)
            grp = (t_steps - 1) // RN
            w = (t_steps - grp * RN) * BL
            nc.scalar.copy(rstrip_t[:, grp * RN * BL:grp * RN * BL + w],
                           rps[:, 0:w])
            flo = ((t_steps - 1) // RN) * RN * BL
            nc.sync.dma_start(rstrip_d[:, flo:], rstrip_t[:, flo:])
            if t_steps <= 6:
                nc.sync.dma_start(rinv_d[:, :], rinv_t[:, :])

    nc.compile()
    return nc


def _get_compiled(t_steps=T):
    if t_steps not in _compiled:
        _compiled[t_steps] = build(t_steps)
    return _compiled[t_steps]


def _host_prep(obs, emis, tran, priors, t_steps):
    """Returns (shared_inputs, per_core_idx, kappa)."""
    nblk = t_steps // TBLK
    # transition softmax -> bf16 chunk layout [j, (jc*HCN+kc)*128 + k]
    m = tran.max(axis=1, keepdims=True)
    e = np.exp(tran - m, dtype=np.float32)
    P = (e / e.sum(axis=1, keepdims=True)).astype(ml_dtypes.bfloat16)
    pm = np.ascontiguousarray(
        P.reshape(HCN, P_, HCN, P_).transpose(1, 0, 2, 3).reshape(P_, -1))

    # transposed bf16 emission table, rows indexed by s*V+v
    tabT = np.ascontiguousarray(
        emis.transpose(0, 2, 1)).astype(ml_dtypes.bfloat16).reshape(S * V, H)

    # L[h] and kappa
    mx = emis.max(axis=2)                                   # (S,H)
    lse = mx + np.log(np.exp(emis - mx[:, :, None],
                             dtype=np.float32).sum(axis=2))
    L = 0.25 * lse.sum(axis=0)                              # (H,)
    kap_h = 0.25 * mx.sum(axis=0) - L
    kappa = float(kap_h.max())
    bias = np.ascontiguousarray(
        (-(L + kappa)).astype(np.float32).reshape(HCN, P_).T)   # (128,4)
    expp = np.ascontiguousarray(
        np.exp(priors, dtype=np.float32).reshape(HCN, P_).T)

    # per-core gather row indices: idx[p=(tt*BL+bb), s*nblk+blk]
    per_core_idx = []
    svec = (np.arange(S, dtype=np.int64) * V)
    for c in range(NC):
        o = obs[c * BL:(c + 1) * BL, :t_steps, :]           # (BL,t,S)
        o = o + svec[None, None, :]
        o = o.transpose(1, 0, 2)                            # (t, BL, S)
        o = o.reshape(nblk, TBLK, BL, S)
        o = o.transpose(1, 2, 3, 0).reshape(TBLK * BL, S * nblk)
        per_core_idx.append(np.ascontiguousarray(o.astype(np.int32)))

    shared = {"tabt": tabT, "pm": pm, "bias": bias, "expp": expp}
    return shared, per_core_idx, kappa


def _host_post(results, lengths, kappa, t_steps):
    nrn = max(1, _n_renorms(t_steps))
    ans = np.zeros((B, 1), np.float32)
    tt = np.arange(t_steps, dtype=np.float64)
    for c in range(NC):
        r = results[c]["rstrip"].reshape(t_steps, BL).astype(np.float64)
        rinv = results[c]["rinvstrip"].reshape(nrn, BL).astype(np.float64)
        rho_log = np.zeros((t_steps, BL), np.float64)
        k = 0
        for t in range(1, t_steps):
            if t % RN == 0:
                rho_log[t] = np.log(rinv[k])
                k += 1
        logsums = np.log(r) + (tt[:, None] + 1.0) * kappa \
            - np.cumsum(rho_log, axis=0)
        lens = np.clip(lengths[c * BL:(c + 1) * BL], 1, t_steps)
        ans[c * BL:(c + 1) * BL, 0] = logsums[
            lens - 1, np.arange(BL)].astype(np.float32)
    return ans


def run(inputs, t_steps=T, trace=False):
    obs = np.asarray(inputs["obs"])
    lengths = np.asarray(inputs["lengths"])
    emis = np.asarray(inputs["unnormalized_emis"], np.float32)
    tran = np.asarray(inputs["unnormalized_tran"], np.float32)
    priors = np.asarray(inputs["log_state_priors"], np.float32)

    nc = _get_compiled(t_steps)
    shared, per_core_idx, kappa = _host_prep(obs, emis, tran, priors, t_steps)
    in_maps = [dict(shared, idx=per_core_idx[c]) for c in range(NC)]
    res = bass_utils.run_bass_kernel_spmd(nc, in_maps,
                                          core_ids=list(range(NC)),
                                          trace=trace)
    ans = _host_post(res.results, lengths, kappa, t_steps)
    return ans, res


def kernel(obs, lengths, unnormalized_emis, unnormalized_tran,
           log_state_priors):
    ans, _ = run(dict(obs=obs, lengths=lengths,
                      unnormalized_emis=unnormalized_emis,
                      unnormalized_tran=unnormalized_tran,
                      log_state_priors=log_state_priors))
    return ans
